# revision 13
# baseline (speedup 1.0000x reference)
"""Trainium2 Bass kernel for nn_Decoder (LSTM decoder w/ attention).

Sharding: 8-way model parallel over hidden dim D for the recurrence
(each core owns 128 of 1024 dims = all 4 gates for those dims), vocab
shard (4000 rows/core) for the output projection, which runs as a
batched matmul over all T*B rows (partly interleaved into the
recurrence's collective stalls, rest at the end).

Matmul operands are bf16 (1 PE cycle/row vs 4 for fp32); PSUM
accumulation and all pointwise state math stay fp32. LSTM sigmoids are
computed via tanh (i,f,o weight rows pre-scaled by 0.5 on the host,
affine-corrected on the vector engine) so the scalar engine only ever
needs the {Tanh, Exp} activation tables -> no per-step table reloads.

Attention logits / context use per-batch-column matmuls (lhsT varies
per b, N=1) writing straight into PSUM, replacing full-product +
DRAM diagonal-extract round trips.

Self-contained: host-side numpy does layout only (transposes, shard
slicing, embedding gather); all FLOPs run on device.
"""

import numpy as np
import ml_dtypes
import bass_rust
import concourse.bass as bass  # noqa: F401  (bass types used via bacc)
import concourse.tile as tile
from concourse import bacc, mybir
from concourse.bass_utils import run_bass_kernel_spmd
from concourse.masks import make_identity

V, E, D = 32000, 512, 1024
TWO_E = 1024
B, S, T = 32, 64, 48
P = 8
DSH = D // P        # 128 hidden dims per core
VSH = V // P        # 4000 vocab rows per core
FP = mybir.dt.float32
BF = mybir.dt.bfloat16
AF = mybir.ActivationFunctionType
ALU = mybir.AluOpType
RG = [list(range(P))]
X = mybir.AxisListType.X
BF_NP = ml_dtypes.bfloat16

# gates0 lhsT layout: [h2 (8x128) | word (4x128) | ones/bias (128) | av (8x128)]
NK0 = 21
# gates1 lhsT layout: [h1 (8x128) | ones/bias (128)]
NK1 = 9


def _build(t_steps=T):
    nc = bacc.Bacc("TRN2", target_bir_lowering=False, debug=False, num_devices=P)
    CW = t_steps * 32  # avhist block width (cols = t*32+b)

    w0s_p = nc.declare_dram_parameter("w0s", [128, NK0 * 512], BF, isOutput=False)
    w1s_p = nc.declare_dram_parameter("w1s", [128, NK1 * 512], BF, isOutput=False)
    wcs_p = nc.declare_dram_parameter("wcs", [128, 16 * 128], BF, isOutput=False)
    wot_p = nc.declare_dram_parameter("wot", [1024, VSH], BF, isOutput=False)
    wpt_p = nc.declare_dram_parameter("wpt", [128, 8 * 128], FP, isOutput=False)
    enct_p = nc.declare_dram_parameter("enct", [1024, 2048], FP, isOutput=False)
    encse_p = nc.declare_dram_parameter("encse", [64, 32 * 128], BF, isOutput=False)
    wordt_p = nc.declare_dram_parameter("wordt", [128, t_steps * 128], BF, isOutput=False)
    h0t_p = nc.declare_dram_parameter("h0t", [128, 8 * 32], BF, isOutput=False)
    c0s_p = nc.declare_dram_parameter("c0s", [32, 128], FP, isOutput=False)
    # scores stored transposed: [vocab_shard, t*32+b]
    out_p = nc.declare_dram_parameter("out", [VSH, CW], FP, isOutput=True)

    with tile.TileContext(nc) as tc:
        with (
            tc.tile_pool(name="res", bufs=1) as res,
            tc.tile_pool(name="wk", bufs=2) as wk,
            tc.tile_pool(name="wop", bufs=8) as wop,
            tc.tile_pool(name="ps1", bufs=1, space="PSUM") as ps1,
            tc.tile_pool(name="ps2", bufs=2, space="PSUM") as ps2,
            tc.tile_pool(name="dr", bufs=2, space="DRAM") as dr,
        ):
            # ---- resident SBUF ----
            w0s = res.tile([128, NK0 * 512], BF, tag="w0s")
            w1s = res.tile([128, NK1 * 512], BF, tag="w1s")
            wcs = res.tile([128, 16 * 128], BF, tag="wcs")
            at = res.tile([128, 2048], FP, tag="at")
            encse = res.tile([64, 32 * 128], BF, tag="encse")
            avhist = res.tile([128, 8 * CW], BF, tag="avhist")
            h1full = res.tile([128, 8 * 32], BF, tag="h1full")
            h2full = res.tile([128, 8 * 32], BF, tag="h2full")
            c = res.tile([32, 128], FP, tag="c")
            ones = res.tile([128, 32], BF, tag="ones")
            id32 = res.tile([32, 32], FP, tag="id32")
            id64 = res.tile([64, 64], FP, tag="id64")
            wpt = res.tile([128, 8 * 128], FP, tag="wpt")

            # ---- init loads (split for overlap) ----
            for kk in range(NK0):
                nc.sync.dma_start(out=w0s[:, 512 * kk:512 * (kk + 1)],
                                  in_=w0s_p[:, 512 * kk:512 * (kk + 1)])
            for kk in range(NK1):
                nc.sync.dma_start(out=w1s[:, 512 * kk:512 * (kk + 1)],
                                  in_=w1s_p[:, 512 * kk:512 * (kk + 1)])
            nc.sync.dma_start(out=wcs[:], in_=wcs_p[:])
            nc.sync.dma_start(out=encse[:], in_=encse_p[:])
            nc.sync.dma_start(out=h2full[:], in_=h0t_p[:])
            nc.sync.dma_start(out=c[:], in_=c0s_p[:])
            nc.sync.dma_start(out=wpt[:], in_=wpt_p[:])

            nc.vector.memset(ones[:], 0.0)
            nc.vector.memset(ones[0:1, :], 1.0)
            make_identity(nc, id32[:])
            make_identity(nc, id64[:])

            # ---- attention scores AT_shard = Wp_shard @ encT ----
            at_ps = [
                ps2.tile([128, 512], FP, tag="mm", name="atps_0"),
                ps2.tile([128, 512], FP, tag="mm", name="atps_1"),
                ps2.tile([128, 512], FP, tag="tr", name="atps_2"),
                ps1.tile([128, 512], FP, tag="av", name="atps_3"),
            ]
            for kk in range(8):
                et = wk.tile([128, 2048], FP, tag="enct", bufs=1)
                nc.sync.dma_start(out=et[:], in_=enct_p[128 * kk:128 * (kk + 1), :])
                for nch in range(4):
                    nc.tensor.matmul(at_ps[nch][:],
                                     wpt[:, 128 * kk:128 * (kk + 1)],
                                     et[:, 512 * nch:512 * (nch + 1)],
                                     start=(kk == 0), stop=(kk == 7))
            for nch in range(4):
                nc.scalar.activation(at[:, 512 * nch:512 * (nch + 1)],
                                     at_ps[nch][:], AF.Copy)

            # output projection (transposed): outT[v, (t,b)] += WoT.T @ av
            wot_ap = wot_p[:]

            pend_stores = []

            def _load_pb(n, vt):
                mv = min(128, VSH - 128 * vt)
                wt_ = wop.tile([128, 8 * 128], BF, tag="wo",
                               name=f"wo_{n}_{vt}")
                # one strided DMA: wt_[p, (j, c)] = wot_p[j*128+p, 128*vt+c]
                src = bass_rust.AP(wot_ap.tensor, wot_ap.offset + 128 * vt,
                                   [[VSH, 128], [128 * VSH, 8], [1, mv]])
                nc.gpsimd.dma_start(
                    out=wt_[:].rearrange("p (j c) -> p j c", c=128)[:, :, 0:mv],
                    in_=src)
                return (n, vt, wt_)

            def _mm_pb(n, vt, wt_, width=256):
                base = 256 * n
                mv = min(128, VSH - 128 * vt)
                bp = ps2.tile([mv, width], FP, tag="g0", name=f"pb_{n}_{vt}")
                for j in range(8):
                    nc.tensor.matmul(
                        bp[:], wt_[:, 128 * j:128 * j + mv],
                        avhist[:, j * CW + base:j * CW + base + width],
                        start=(j == 0), stop=(j == 7))
                bs_ = wk.tile([mv, width], FP, tag="bstg", name=f"pbs_{n}_{vt}")
                nc.vector.tensor_copy(bs_[:], bp[:])
                pend_stores.append((n, vt, mv, width, bs_))

            def _flush_stores():
                for n, vt, mv, width, bs_ in pend_stores:
                    nc.sync.dma_start(
                        out=out_p[128 * vt:128 * vt + mv,
                                  256 * n:256 * n + width],
                        in_=bs_[:])
                del pend_stores[:]

            def _lstm_pointwise(g, pfx):
                # g: [32, 512] PSUM, cols [i|f|o|g]; i,f,o pre-scaled by 0.5.
                th = wk.tile([32, 512], FP, tag="th", name=f"{pfx}_th")
                nc.scalar.activation(th[:], g[:], AF.Tanh)
                sifo = wk.tile([32, 384], FP, tag="sifo", name=f"{pfx}_sf")
                nc.vector.tensor_scalar(sifo[:], th[:, 0:384], 0.5, 0.5,
                                        ALU.mult, ALU.add)
                t1 = wk.tile([32, 128], FP, tag="t1", name=f"{pfx}_t1")
                t2 = wk.tile([32, 128], FP, tag="t2", name=f"{pfx}_t2")
                nc.vector.tensor_mul(t1[:], sifo[:, 128:256], c[:])
                nc.vector.tensor_mul(t2[:], sifo[:, 0:128], th[:, 384:512])
                nc.vector.tensor_add(c[:], t1[:], t2[:])
                tc1 = wk.tile([32, 128], FP, tag="tc1", name=f"{pfx}_tc")
                nc.scalar.activation(tc1[:], c[:], AF.Tanh)
                h = wk.tile([32, 128], FP, tag="h", name=f"{pfx}_h")
                nc.vector.tensor_mul(h[:], sifo[:, 256:384], tc1[:])
                return h

            # ---- recurrence ----
            for t in range(t_steps):
                # projection work for this step: weights prefetched on the
                # idle gpsimd queue now, matmuls spread across the three
                # collective windows (keeps the PE busy -> HAM stays warm),
                # stores flushed at step end on the sync queue.
                if t_steps == 48 and t >= 8:
                    _n = (t - 8) // 8
                    _bv = 4 * ((t - 8) % 8)
                    pend = [_load_pb(_n, _bv + q) for q in range(4)]
                else:
                    pend = []

                # gates0: g0 = W0 @ [h2; word; 1; av]  (av part last: it
                # lands after this step's AllReduce; the rest runs during it)
                g0 = ps2.tile([32, 512], FP, tag="g0")
                word = wk.tile([128, 128], BF, tag="word")
                nc.gpsimd.dma_start(out=word[:], in_=wordt_p[:, 128 * t:128 * (t + 1)])
                mms = []
                for j in range(8):
                    mms.append((h2full[:, 32 * j:32 * (j + 1)], j))
                for j in range(4):
                    mms.append((word[:, 32 * j:32 * (j + 1)], 8 + j))
                mms.append((ones[:], 12))
                if t > 0:
                    for j in range(8):
                        mms.append((avhist[:, j * CW + 32 * (t - 1):
                                           j * CW + 32 * t], 13 + j))
                for i, (lhsT, kk) in enumerate(mms):
                    nc.tensor.matmul(g0[:], lhsT,
                                     w0s[:, 512 * kk:512 * (kk + 1)],
                                     start=(i == 0), stop=(i == len(mms) - 1))

                h1 = _lstm_pointwise(g0, "c0")

                # h1 -> h1T shard, AllGather -> h1full
                trp = ps2.tile([128, 32], FP, tag="tr")
                nc.tensor.transpose(trp[:], h1[:], id32[:])
                h1t = wk.tile([128, 32], BF, tag="h1t")
                nc.scalar.activation(h1t[:], trp[:], AF.Copy)
                b1 = dr.tile([128, 32], BF, tag="b1")
                o1 = dr.tile([1024, 32], BF, tag="o1")
                nc.sync.dma_start(out=b1[:], in_=h1t[:])
                nc.gpsimd.collective_compute(
                    "AllGather", ALU.bypass,
                    replica_groups=RG, ins=[b1.opt()], outs=[o1.opt()])
                for _ in range(2):
                    if pend:
                        _mm_pb(*pend.pop(0))
                o1_ap = o1[:]
                nc.sync.dma_start(
                    out=h1full[:].rearrange("p (j b) -> p j b", b=32),
                    in_=bass_rust.AP(o1_ap.tensor, o1_ap.offset,
                                     [[32, 128], [4096, 8], [1, 32]]))

                # gates1: g1 = W1 @ [h1; 1]
                g1 = ps2.tile([32, 512], FP, tag="mm")
                for j in range(8):
                    nc.tensor.matmul(g1[:], h1full[:, 32 * j:32 * (j + 1)],
                                     w1s[:, 512 * j:512 * (j + 1)],
                                     start=(j == 0), stop=False)
                nc.tensor.matmul(g1[:], ones[:], w1s[:, 512 * 8:512 * 9],
                                 start=False, stop=True)

                h2 = _lstm_pointwise(g1, "c1")

                # h2 -> h2T shard
                trp2 = ps2.tile([128, 32], FP, tag="tr")
                nc.tensor.transpose(trp2[:], h2[:], id32[:])
                h2t = wk.tile([128, 32], BF, tag="h2t")
                nc.scalar.activation(h2t[:], trp2[:], AF.Copy)
                h2tf = wk.tile([128, 32], FP, tag="h2tf")
                nc.vector.tensor_copy(h2tf[:], trp2[:])

                # logits partial (own 128 d-dims), per-b columns:
                # lgT[s, b] = sum_d at[d, 64b+s] * h2t[d, b]
                lg_ps = ps2.tile([64, 32], FP, tag="tr")
                for b in range(32):
                    nc.tensor.matmul(lg_ps[:, b:b + 1],
                                     at[:, 64 * b:64 * (b + 1)],
                                     h2tf[:, b:b + 1], start=True, stop=True)
                lgt = wk.tile([64, 32], FP, tag="lgt")
                nc.scalar.activation(lgt[:], lg_ps[:], AF.Copy)

                # merged AllGather #2: [h2T shard (128x32 bf16) | logits
                # partial (64x32 fp32, bit-carried as bf16 rows 128:256)]
                bm = dr.tile([256, 32], BF, tag="bm")
                om = dr.tile([2048, 32], BF, tag="om")
                nc.sync.dma_start(out=bm[0:128, :], in_=h2t[:])
                bm_fp = bm[:].bitcast(FP)
                nc.sync.dma_start(
                    out=bass_rust.AP(bm_fp.tensor, bm_fp.offset + 2048,
                                     [[32, 64], [16, 2], [1, 16]]),
                    in_=lgt[:].rearrange("p (a e) -> p a e", e=16))
                nc.gpsimd.collective_compute(
                    "AllGather", ALU.bypass,
                    replica_groups=RG, ins=[bm.opt()], outs=[om.opt()])
                for _ in range(2):
                    if pend:
                        _mm_pb(*pend.pop(0))
                om_ap = om[:]
                # logits partials first (softmax chain is the critical path)
                om_fp = om[:].bitcast(FP)
                ls = wk.tile([64, 8 * 32], FP, tag="ls")
                nc.sync.dma_start(
                    out=ls[:].rearrange("p (j b) -> p j b", b=32),
                    in_=bass_rust.AP(om_fp.tensor, om_fp.offset + 2048,
                                     [[32, 64], [4096, 8], [1, 32]]))
                # h2full[p, j*32+b] = om[j*256 + p, b]
                nc.sync.dma_start(
                    out=h2full[:].rearrange("p (j b) -> p j b", b=32),
                    in_=bass_rust.AP(om_ap.tensor, om_ap.offset,
                                     [[32, 128], [8192, 8], [1, 32]]))
                lgsum = wk.tile([64, 32], FP, tag="lgsum")
                nc.vector.reduce_sum(
                    lgsum[:], ls[:].rearrange("p (j b) -> p b j", b=32),
                    axis=X)

                # transpose -> [b, s], softmax over s
                trl = ps2.tile([32, 64], FP, tag="tr", name=f"trl_{t}")
                nc.tensor.transpose(trl[:], lgsum[:], id64[:])
                lg0 = wk.tile([32, 64], FP, tag="lgs")
                nc.scalar.activation(lg0[:], trl[:], AF.Copy)
                mx = wk.tile([32, 1], FP, tag="mx")
                nc.vector.reduce_max(mx[:], lg0[:], axis=X, negate=True)
                ex = wk.tile([32, 64], FP, tag="ex")
                nc.scalar.activation(ex[:], lg0[:], AF.Exp, bias=mx[:])
                sm = wk.tile([32, 1], FP, tag="sm")
                nc.vector.reduce_sum(sm[:], ex[:], axis=X)
                rc = wk.tile([32, 1], FP, tag="rc")
                nc.vector.reciprocal(rc[:], sm[:])
                al = wk.tile([32, 64], FP, tag="al")
                nc.vector.tensor_scalar_mul(al[:], ex[:], rc[:])

                # alpha -> alphaT
                trp3 = ps2.tile([64, 32], FP, tag="tr")
                nc.tensor.transpose(trp3[:], al[:], id32[:])
                alt = wk.tile([64, 32], BF, tag="alt")
                nc.scalar.activation(alt[:], trp3[:], AF.Copy)

                # context (own 128 e-dims), per-b columns:
                # cxt[e, b] = sum_s encse[s, 128b+e] * alt[s, b]
                cxt_ps = ps2.tile([128, 32], FP, tag="tr")
                for b in range(32):
                    nc.tensor.matmul(cxt_ps[:, b:b + 1],
                                     encse[:, 128 * b:128 * (b + 1)],
                                     alt[:, b:b + 1], start=True, stop=True)
                cxt = wk.tile([128, 32], BF, tag="cxt")
                nc.scalar.activation(cxt[:], cxt_ps[:], AF.Copy)

                # combine: av_preT[m-dims, b] over own 256 K dims
                # (contiguous accumulation group: interleaving other matmuls
                # inside an open PSUM group corrupts it on HW)
                avp = ps1.tile([128, 256], FP, tag="av")
                for m in range(8):
                    nc.tensor.matmul(avp[:, 32 * m:32 * (m + 1)],
                                     wcs[:, m * 128:m * 128 + 128],
                                     h2t[:], start=True, stop=False)
                    nc.tensor.matmul(avp[:, 32 * m:32 * (m + 1)],
                                     wcs[:, (8 + m) * 128:(8 + m) * 128 + 128],
                                     cxt[:], start=False, stop=True)
                avs = wk.tile([128, 256], BF, tag="avs")
                nc.scalar.activation(avs[:], avp[:], AF.Copy)
                bav = dr.tile([1024, 32], BF, tag="bav")
                oav = dr.tile([1024, 32], BF, tag="oav")
                nc.sync.dma_start(
                    out=bav[:].rearrange("(m p) b -> p m b", p=128),
                    in_=avs[:].rearrange("p (m b) -> p m b", b=32))
                nc.gpsimd.collective_compute(
                    "AllReduce", ALU.add,
                    replica_groups=RG, ins=[bav.opt()], outs=[oav.opt()])
                while pend:
                    _mm_pb(*pend.pop(0))
                avpre = wk.tile([128, 256], BF, tag="avpre")
                nc.sync.dma_start(
                    out=avpre[:].rearrange("p (j b) -> p j b", b=32),
                    in_=oav[:].rearrange("(j p) b -> p j b", p=128))
                # av = tanh(av_pre), written into avhist column group t
                dst = avhist[:].rearrange("p (j cc) -> p j cc", cc=CW)[:, :, 32 * t:32 * (t + 1)]
                nc.scalar.activation(dst, avpre[:].rearrange("p (j b) -> p j b", b=32),
                                     AF.Tanh)
                _flush_stores()

            # ---- remaining output-projection chunks ----
            if t_steps == 48:
                for vt in range(32):
                    _mm_pb(*_load_pb(5, vt))
                    _flush_stores()
            else:
                nch = (CW + 255) // 256
                for n in range(nch):
                    w = min(256, CW - 256 * n)
                    for vt in range(32):
                        _mm_pb(*_load_pb(n, vt), width=w)
                        _flush_stores()

    nc.compile()
    return nc


def _prep(inputs, t_steps=T):
    g = {k: np.asarray(v) for k, v in inputs.items()}
    src = g["src_encodings"].astype(np.float32)          # [S, B, 2E]
    h0 = g["h0"].astype(np.float32)
    c0 = g["c0"].astype(np.float32)
    emb = g["embedding"].astype(np.float32)
    Wp = g["W_proj"].astype(np.float32)
    Wc = g["W_combine"].astype(np.float32)
    Wo = g["W_out"].astype(np.float32)
    Wih0 = g["W_ih0"].astype(np.float32)
    Whh0 = g["W_hh0"].astype(np.float32)
    bih0 = g["b_ih0"].astype(np.float32)
    bhh0 = g["b_hh0"].astype(np.float32)
    Wih1 = g["W_ih1"].astype(np.float32)
    Whh1 = g["W_hh1"].astype(np.float32)
    bih1 = g["b_ih1"].astype(np.float32)
    bhh1 = g["b_hh1"].astype(np.float32)
    tgt = np.asarray(g["tgt_tensor"]).astype(np.int64)   # [T, B]

    W1 = Wih1 + Whh1
    b0 = bih0 + bhh0
    b1 = bih1 + bhh1

    # shared across cores
    wemb = emb[tgt[:t_steps]]                            # [t, B, E]
    # wordt: [128, t*128]; step block t = wordT[:,t] split into 4 j-blocks
    wordt = (wemb.transpose(0, 2, 1)                     # [t, E, B]
             .reshape(t_steps, 4, 128, 32)
             .transpose(2, 0, 1, 3).reshape(128, t_steps * 128))
    wordt = np.ascontiguousarray(wordt).astype(BF_NP)
    enct = np.ascontiguousarray(
        src.transpose(2, 1, 0).reshape(1024, 2048))  # [e, b*64+s]
    h0t = np.ascontiguousarray(
        h0.T.reshape(8, 128, 32).transpose(1, 0, 2).reshape(128, 256)
    ).astype(BF_NP)

    in_maps = []
    for k in range(P):
        rows = np.concatenate([gg * 1024 + k * 128 + np.arange(128)
                               for gg in (0, 1, 3, 2)])  # [i|f|o|g] x 128 dims
        # W0sT_aug rows: [h2 1024 | word 512 | bias 1 | pad | av 1024]
        w0a = np.zeros((NK0 * 128, 512), np.float32)
        w0a[0:1024] = Whh0[rows].T
        w0a[1024:1536] = Wih0[rows, 0:512].T
        w0a[1536] = b0[rows]
        w0a[1664:2688] = Wih0[rows, 512:1536].T
        w0a[:, 0:384] *= 0.5   # i,f,o via tanh: sigma(x) = 0.5*tanh(x/2)+0.5
        w0s = np.ascontiguousarray(
            w0a.reshape(NK0, 128, 512).transpose(1, 0, 2).reshape(128, NK0 * 512)
        ).astype(BF_NP)

        w1a = np.zeros((NK1 * 128, 512), np.float32)
        w1a[0:1024] = W1[rows].T
        w1a[1024] = b1[rows]
        w1a[:, 0:384] *= 0.5
        w1s = np.ascontiguousarray(
            w1a.reshape(NK1, 128, 512).transpose(1, 0, 2).reshape(128, NK1 * 512)
        ).astype(BF_NP)

        # Wc own-K slice: h dims [128k..] and ctx dims [1024+128k..]
        hs = slice(k * 128, k * 128 + 128)
        cs = slice(1024 + k * 128, 1024 + k * 128 + 128)
        wc_own = np.concatenate([Wc[:, hs], Wc[:, cs]], axis=1)  # [1024, 256]
        blocks = []
        for j in range(2):
            for m in range(8):
                blocks.append(wc_own[128 * m:128 * (m + 1),
                                     128 * j:128 * (j + 1)].T)
        wcs = np.ascontiguousarray(
            np.concatenate(blocks, axis=1)).astype(BF_NP)  # [128, 16*128]

        wot = np.ascontiguousarray(
            Wo[VSH * k:VSH * (k + 1)].T).astype(BF_NP)     # [1024, 4000]
        wpt_ = Wp[128 * k:128 * (k + 1), :].T              # [1024, 128]
        wpt = np.ascontiguousarray(
            wpt_.reshape(8, 128, 128).transpose(1, 0, 2).reshape(128, 8 * 128))
        # encse[s, b*128 + e] = src[s, b, e_shard]
        encse = np.ascontiguousarray(
            src[:, :, 128 * k:128 * (k + 1)].reshape(64, 32 * 128)).astype(BF_NP)
        c0s = np.ascontiguousarray(c0[:, 128 * k:128 * (k + 1)])

        in_maps.append({
            "w0s": w0s, "w1s": w1s, "wcs": wcs, "wot": wot, "wpt": wpt,
            "enct": enct, "encse": encse, "wordt": wordt,
            "h0t": h0t, "c0s": c0s,
        })
    return in_maps


_CACHE = {}


def _get_nc(t_steps=T):
    if t_steps not in _CACHE:
        _CACHE[t_steps] = _build(t_steps)
    return _CACHE[t_steps]


def run_device(inputs, trace=False, t_steps=T):
    nc = _get_nc(t_steps)
    in_maps = _prep(inputs, t_steps)
    return run_bass_kernel_spmd(nc, in_maps, core_ids=list(range(P)), trace=trace)


def assemble(results, t_steps=T):
    return np.concatenate(
        [np.ascontiguousarray(np.asarray(results[k]["out"]).T)
         .reshape(t_steps, B, VSH) for k in range(P)],
        axis=2)


def kernel(**inputs):
    r = run_device(inputs)
    return assemble(r.results)


# revision 15
# speedup vs baseline: 1.0020x; 1.0020x over previous
"""Trainium2 Bass kernel for nn_Decoder (LSTM decoder w/ attention).

Sharding: 8-way model parallel over hidden dim D for the recurrence
(each core owns 128 of 1024 dims = all 4 gates for those dims), vocab
shard (4000 rows/core) for the output projection, which runs as a
batched matmul over all T*B rows (partly interleaved into the
recurrence's collective stalls, rest at the end).

Matmul operands are bf16 (1 PE cycle/row vs 4 for fp32); PSUM
accumulation and all pointwise state math stay fp32. LSTM sigmoids are
computed via tanh (i,f,o weight rows pre-scaled by 0.5 on the host,
affine-corrected on the vector engine) so the scalar engine only ever
needs the {Tanh, Exp} activation tables -> no per-step table reloads.

Attention logits / context use per-batch-column matmuls (lhsT varies
per b, N=1) writing straight into PSUM, replacing full-product +
DRAM diagonal-extract round trips.

Self-contained: host-side numpy does layout only (transposes, shard
slicing, embedding gather); all FLOPs run on device.
"""

import numpy as np
import ml_dtypes
import bass_rust
import concourse.bass as bass  # noqa: F401  (bass types used via bacc)
import concourse.tile as tile
from concourse import bacc, mybir
from concourse.bass_utils import run_bass_kernel_spmd
from concourse.masks import make_identity

V, E, D = 32000, 512, 1024
TWO_E = 1024
B, S, T = 32, 64, 48
P = 8
DSH = D // P        # 128 hidden dims per core
VSH = V // P        # 4000 vocab rows per core
FP = mybir.dt.float32
BF = mybir.dt.bfloat16
AF = mybir.ActivationFunctionType
ALU = mybir.AluOpType
RG = [list(range(P))]
X = mybir.AxisListType.X
BF_NP = ml_dtypes.bfloat16

# gates0 lhsT layout: [h2 (8x128) | word (4x128) | ones/bias (128) | av (8x128)]
NK0 = 21
# gates1 lhsT layout: [h1 (8x128) | ones/bias (128)]
NK1 = 9


def _build(t_steps=T):
    nc = bacc.Bacc("TRN2", target_bir_lowering=False, debug=False, num_devices=P)
    CW = t_steps * 32  # avhist block width (cols = t*32+b)

    w0s_p = nc.declare_dram_parameter("w0s", [128, NK0 * 512], BF, isOutput=False)
    w1s_p = nc.declare_dram_parameter("w1s", [128, NK1 * 512], BF, isOutput=False)
    wcs_p = nc.declare_dram_parameter("wcs", [128, 16 * 128], BF, isOutput=False)
    wot_p = nc.declare_dram_parameter("wot", [1024, VSH], BF, isOutput=False)
    wpt_p = nc.declare_dram_parameter("wpt", [128, 8 * 128], FP, isOutput=False)
    enct_p = nc.declare_dram_parameter("enct", [1024, 2048], FP, isOutput=False)
    encse_p = nc.declare_dram_parameter("encse", [64, 32 * 128], BF, isOutput=False)
    wordt_p = nc.declare_dram_parameter("wordt", [128, t_steps * 128], BF, isOutput=False)
    h0t_p = nc.declare_dram_parameter("h0t", [128, 8 * 32], BF, isOutput=False)
    c0s_p = nc.declare_dram_parameter("c0s", [32, 128], FP, isOutput=False)
    # scores stored transposed: [vocab_shard, t*32+b]
    out_p = nc.declare_dram_parameter("out", [VSH, CW], FP, isOutput=True)

    with tile.TileContext(nc) as tc:
        with (
            tc.tile_pool(name="res", bufs=1) as res,
            tc.tile_pool(name="wk", bufs=2) as wk,
            tc.tile_pool(name="wop", bufs=8) as wop,
            tc.tile_pool(name="ps1", bufs=1, space="PSUM") as ps1,
            tc.tile_pool(name="ps2", bufs=2, space="PSUM") as ps2,
            tc.tile_pool(name="dr", bufs=2, space="DRAM") as dr,
        ):
            # ---- resident SBUF ----
            w0s = res.tile([128, NK0 * 512], BF, tag="w0s")
            w1s = res.tile([128, NK1 * 512], BF, tag="w1s")
            wcs = res.tile([128, 16 * 128], BF, tag="wcs")
            at = res.tile([128, 2048], FP, tag="at")
            encse = res.tile([64, 32 * 128], BF, tag="encse")
            avhist = res.tile([128, 8 * CW], BF, tag="avhist")
            h1full = res.tile([128, 8 * 32], BF, tag="h1full")
            h2full = res.tile([128, 8 * 32], BF, tag="h2full")
            c = res.tile([32, 128], FP, tag="c")
            ones = res.tile([128, 32], BF, tag="ones")
            id32 = res.tile([32, 32], FP, tag="id32")
            id64 = res.tile([64, 64], FP, tag="id64")
            wpt = res.tile([128, 8 * 128], FP, tag="wpt")

            # ---- init loads (split for overlap) ----
            for kk in range(NK0):
                nc.sync.dma_start(out=w0s[:, 512 * kk:512 * (kk + 1)],
                                  in_=w0s_p[:, 512 * kk:512 * (kk + 1)])
            for kk in range(NK1):
                nc.sync.dma_start(out=w1s[:, 512 * kk:512 * (kk + 1)],
                                  in_=w1s_p[:, 512 * kk:512 * (kk + 1)])
            nc.sync.dma_start(out=wcs[:], in_=wcs_p[:])
            nc.sync.dma_start(out=encse[:], in_=encse_p[:])
            nc.sync.dma_start(out=h2full[:], in_=h0t_p[:])
            nc.sync.dma_start(out=c[:], in_=c0s_p[:])
            nc.sync.dma_start(out=wpt[:], in_=wpt_p[:])

            nc.vector.memset(ones[:], 0.0)
            nc.vector.memset(ones[0:1, :], 1.0)
            make_identity(nc, id32[:])
            make_identity(nc, id64[:])

            # ---- attention scores AT_shard = Wp_shard @ encT ----
            at_ps = [
                ps2.tile([128, 512], FP, tag="mm", name="atps_0"),
                ps2.tile([128, 512], FP, tag="mm", name="atps_1"),
                ps2.tile([128, 512], FP, tag="tr", name="atps_2"),
                ps1.tile([128, 512], FP, tag="av", name="atps_3"),
            ]
            for kk in range(8):
                et = wk.tile([128, 2048], FP, tag="enct", bufs=1)
                nc.sync.dma_start(out=et[:], in_=enct_p[128 * kk:128 * (kk + 1), :])
                for nch in range(4):
                    nc.tensor.matmul(at_ps[nch][:],
                                     wpt[:, 128 * kk:128 * (kk + 1)],
                                     et[:, 512 * nch:512 * (nch + 1)],
                                     start=(kk == 0), stop=(kk == 7))
            for nch in range(4):
                nc.scalar.activation(at[:, 512 * nch:512 * (nch + 1)],
                                     at_ps[nch][:], AF.Copy)

            # output projection (transposed): outT[v, (t,b)] += WoT.T @ av
            wot_ap = wot_p[:]

            pend_stores = []

            def _load_pb(n, vt):
                mv = min(128, VSH - 128 * vt)
                wt_ = wop.tile([128, 8 * 128], BF, tag="wo",
                               name=f"wo_{n}_{vt}")
                # one strided DMA: wt_[p, (j, c)] = wot_p[j*128+p, 128*vt+c]
                src = bass_rust.AP(wot_ap.tensor, wot_ap.offset + 128 * vt,
                                   [[VSH, 128], [128 * VSH, 8], [1, mv]])
                nc.gpsimd.dma_start(
                    out=wt_[:].rearrange("p (j c) -> p j c", c=128)[:, :, 0:mv],
                    in_=src)
                return (n, vt, wt_)

            def _mm_pb(n, vt, wt_, width=256):
                base = 256 * n
                mv = min(128, VSH - 128 * vt)
                bp = ps2.tile([mv, width], FP, tag="g0", name=f"pb_{n}_{vt}")
                for j in range(8):
                    nc.tensor.matmul(
                        bp[:], wt_[:, 128 * j:128 * j + mv],
                        avhist[:, j * CW + base:j * CW + base + width],
                        start=(j == 0), stop=(j == 7))
                bs_ = wk.tile([mv, width], FP, tag="bstg", name=f"pbs_{n}_{vt}",
                              bufs=4)
                nc.vector.tensor_copy(bs_[:], bp[:])
                pend_stores.append((n, vt, mv, width, bs_))

            def _flush_stores():
                for n, vt, mv, width, bs_ in pend_stores:
                    nc.sync.dma_start(
                        out=out_p[128 * vt:128 * vt + mv,
                                  256 * n:256 * n + width],
                        in_=bs_[:])
                del pend_stores[:]

            def _lstm_pointwise(g, pfx):
                # g: [32, 512] PSUM, cols [i|f|o|g]; i,f,o pre-scaled by 0.5.
                th = wk.tile([32, 512], FP, tag="th", name=f"{pfx}_th")
                nc.scalar.activation(th[:], g[:], AF.Tanh)
                sifo = wk.tile([32, 384], FP, tag="sifo", name=f"{pfx}_sf")
                nc.vector.tensor_scalar(sifo[:], th[:, 0:384], 0.5, 0.5,
                                        ALU.mult, ALU.add)
                t1 = wk.tile([32, 128], FP, tag="t1", name=f"{pfx}_t1")
                t2 = wk.tile([32, 128], FP, tag="t2", name=f"{pfx}_t2")
                nc.vector.tensor_mul(t1[:], sifo[:, 128:256], c[:])
                nc.vector.tensor_mul(t2[:], sifo[:, 0:128], th[:, 384:512])
                nc.vector.tensor_add(c[:], t1[:], t2[:])
                tc1 = wk.tile([32, 128], FP, tag="tc1", name=f"{pfx}_tc")
                nc.scalar.activation(tc1[:], c[:], AF.Tanh)
                h = wk.tile([32, 128], FP, tag="h", name=f"{pfx}_h")
                nc.vector.tensor_mul(h[:], sifo[:, 256:384], tc1[:])
                return h

            # ---- recurrence ----
            for t in range(t_steps):
                # projection work for this step: weights prefetched on the
                # idle gpsimd queue now, matmuls spread across the three
                # collective windows (keeps the PE busy -> HAM stays warm),
                # stores flushed at step end on the sync queue.
                if t_steps == 48 and t >= 8:
                    _n = (t - 8) // 8
                    _bv = 4 * ((t - 8) % 8)
                    pend = [_load_pb(_n, _bv + q) for q in range(4)]
                else:
                    pend = []

                # gates0: g0 = W0 @ [h2; word; 1; av]  (av part last: it
                # lands after this step's AllReduce; the rest runs during it)
                g0 = ps2.tile([32, 512], FP, tag="g0")
                word = wk.tile([128, 128], BF, tag="word")
                nc.gpsimd.dma_start(out=word[:], in_=wordt_p[:, 128 * t:128 * (t + 1)])
                mms = []
                for j in range(8):
                    mms.append((h2full[:, 32 * j:32 * (j + 1)], j))
                for j in range(4):
                    mms.append((word[:, 32 * j:32 * (j + 1)], 8 + j))
                mms.append((ones[:], 12))
                if t > 0:
                    for j in range(8):
                        mms.append((avhist[:, j * CW + 32 * (t - 1):
                                           j * CW + 32 * t], 13 + j))
                for i, (lhsT, kk) in enumerate(mms):
                    nc.tensor.matmul(g0[:], lhsT,
                                     w0s[:, 512 * kk:512 * (kk + 1)],
                                     start=(i == 0), stop=(i == len(mms) - 1))

                h1 = _lstm_pointwise(g0, "c0")

                # h1 -> h1T shard, AllGather -> h1full
                trp = ps2.tile([128, 32], FP, tag="tr")
                nc.tensor.transpose(trp[:], h1[:], id32[:])
                h1t = wk.tile([128, 32], BF, tag="h1t")
                nc.scalar.activation(h1t[:], trp[:], AF.Copy)
                b1 = dr.tile([128, 32], BF, tag="b1")
                o1 = dr.tile([1024, 32], BF, tag="o1")
                nc.sync.dma_start(out=b1[:], in_=h1t[:])
                nc.gpsimd.collective_compute(
                    "AllGather", ALU.bypass,
                    replica_groups=RG, ins=[b1.opt()], outs=[o1.opt()])
                for _ in range(2):
                    if pend:
                        _mm_pb(*pend.pop(0))
                o1_ap = o1[:]
                nc.sync.dma_start(
                    out=h1full[:].rearrange("p (j b) -> p j b", b=32),
                    in_=bass_rust.AP(o1_ap.tensor, o1_ap.offset,
                                     [[32, 128], [4096, 8], [1, 32]]))

                # gates1: g1 = W1 @ [h1; 1]
                g1 = ps2.tile([32, 512], FP, tag="mm")
                for j in range(8):
                    nc.tensor.matmul(g1[:], h1full[:, 32 * j:32 * (j + 1)],
                                     w1s[:, 512 * j:512 * (j + 1)],
                                     start=(j == 0), stop=False)
                nc.tensor.matmul(g1[:], ones[:], w1s[:, 512 * 8:512 * 9],
                                 start=False, stop=True)

                h2 = _lstm_pointwise(g1, "c1")

                # h2 -> h2T shard
                trp2 = ps2.tile([128, 32], FP, tag="tr")
                nc.tensor.transpose(trp2[:], h2[:], id32[:])
                h2t = wk.tile([128, 32], BF, tag="h2t")
                nc.scalar.activation(h2t[:], trp2[:], AF.Copy)
                h2tf = wk.tile([128, 32], FP, tag="h2tf")
                nc.vector.tensor_copy(h2tf[:], trp2[:])

                # logits partial (own 128 d-dims), per-b columns:
                # lgT[s, b] = sum_d at[d, 64b+s] * h2t[d, b]
                lg_ps = ps2.tile([64, 32], FP, tag="tr")
                for b in range(32):
                    nc.tensor.matmul(lg_ps[:, b:b + 1],
                                     at[:, 64 * b:64 * (b + 1)],
                                     h2tf[:, b:b + 1], start=True, stop=True)
                lgt = wk.tile([64, 32], FP, tag="lgt")
                nc.scalar.activation(lgt[:], lg_ps[:], AF.Copy)

                # merged AllGather #2: [h2T shard (128x32 bf16) | logits
                # partial (64x32 fp32, bit-carried as bf16 rows 128:256)]
                bm = dr.tile([256, 32], BF, tag="bm")
                om = dr.tile([2048, 32], BF, tag="om")
                nc.sync.dma_start(out=bm[0:128, :], in_=h2t[:])
                bm_fp = bm[:].bitcast(FP)
                nc.sync.dma_start(
                    out=bass_rust.AP(bm_fp.tensor, bm_fp.offset + 2048,
                                     [[32, 64], [16, 2], [1, 16]]),
                    in_=lgt[:].rearrange("p (a e) -> p a e", e=16))
                nc.gpsimd.collective_compute(
                    "AllGather", ALU.bypass,
                    replica_groups=RG, ins=[bm.opt()], outs=[om.opt()])
                for _ in range(2):
                    if pend:
                        _mm_pb(*pend.pop(0))
                om_ap = om[:]
                # logits partials first (softmax chain is the critical path)
                om_fp = om[:].bitcast(FP)
                ls = wk.tile([64, 8 * 32], FP, tag="ls")
                nc.sync.dma_start(
                    out=ls[:].rearrange("p (j b) -> p j b", b=32),
                    in_=bass_rust.AP(om_fp.tensor, om_fp.offset + 2048,
                                     [[32, 64], [4096, 8], [1, 32]]))
                # h2full[p, j*32+b] = om[j*256 + p, b]
                nc.sync.dma_start(
                    out=h2full[:].rearrange("p (j b) -> p j b", b=32),
                    in_=bass_rust.AP(om_ap.tensor, om_ap.offset,
                                     [[32, 128], [8192, 8], [1, 32]]))
                lgsum = wk.tile([64, 32], FP, tag="lgsum")
                nc.vector.reduce_sum(
                    lgsum[:], ls[:].rearrange("p (j b) -> p b j", b=32),
                    axis=X)

                # transpose -> [b, s], softmax over s
                trl = ps2.tile([32, 64], FP, tag="tr", name=f"trl_{t}")
                nc.tensor.transpose(trl[:], lgsum[:], id64[:])
                lg0 = wk.tile([32, 64], FP, tag="lgs")
                nc.scalar.activation(lg0[:], trl[:], AF.Copy)
                mx = wk.tile([32, 1], FP, tag="mx")
                nc.vector.reduce_max(mx[:], lg0[:], axis=X, negate=True)
                ex = wk.tile([32, 64], FP, tag="ex")
                nc.scalar.activation(ex[:], lg0[:], AF.Exp, bias=mx[:])
                sm = wk.tile([32, 1], FP, tag="sm")
                nc.vector.reduce_sum(sm[:], ex[:], axis=X)
                rc = wk.tile([32, 1], FP, tag="rc")
                nc.vector.reciprocal(rc[:], sm[:])
                al = wk.tile([32, 64], FP, tag="al")
                nc.vector.tensor_scalar_mul(al[:], ex[:], rc[:])

                # alpha -> alphaT
                trp3 = ps2.tile([64, 32], FP, tag="tr")
                nc.tensor.transpose(trp3[:], al[:], id32[:])
                alt = wk.tile([64, 32], BF, tag="alt")
                nc.scalar.activation(alt[:], trp3[:], AF.Copy)

                # context (own 128 e-dims), per-b columns:
                # cxt[e, b] = sum_s encse[s, 128b+e] * alt[s, b]
                cxt_ps = ps2.tile([128, 32], FP, tag="tr")
                for b in range(32):
                    nc.tensor.matmul(cxt_ps[:, b:b + 1],
                                     encse[:, 128 * b:128 * (b + 1)],
                                     alt[:, b:b + 1], start=True, stop=True)
                cxt = wk.tile([128, 32], BF, tag="cxt")
                nc.scalar.activation(cxt[:], cxt_ps[:], AF.Copy)

                # combine: av_preT[m-dims, b] over own 256 K dims
                # (contiguous accumulation group: interleaving other matmuls
                # inside an open PSUM group corrupts it on HW)
                avp = ps1.tile([128, 256], FP, tag="av")
                for m in range(8):
                    nc.tensor.matmul(avp[:, 32 * m:32 * (m + 1)],
                                     wcs[:, m * 128:m * 128 + 128],
                                     h2t[:], start=True, stop=False)
                    nc.tensor.matmul(avp[:, 32 * m:32 * (m + 1)],
                                     wcs[:, (8 + m) * 128:(8 + m) * 128 + 128],
                                     cxt[:], start=False, stop=True)
                avs = wk.tile([128, 256], BF, tag="avs")
                nc.scalar.activation(avs[:], avp[:], AF.Copy)
                bav = dr.tile([1024, 32], BF, tag="bav")
                oav = dr.tile([1024, 32], BF, tag="oav")
                nc.sync.dma_start(
                    out=bav[:].rearrange("(m p) b -> p m b", p=128),
                    in_=avs[:].rearrange("p (m b) -> p m b", b=32))
                nc.gpsimd.collective_compute(
                    "AllReduce", ALU.add,
                    replica_groups=RG, ins=[bav.opt()], outs=[oav.opt()])
                while pend:
                    _mm_pb(*pend.pop(0))
                avpre = wk.tile([128, 256], BF, tag="avpre")
                nc.sync.dma_start(
                    out=avpre[:].rearrange("p (j b) -> p j b", b=32),
                    in_=oav[:].rearrange("(j p) b -> p j b", p=128))
                # av = tanh(av_pre), written into avhist column group t
                dst = avhist[:].rearrange("p (j cc) -> p j cc", cc=CW)[:, :, 32 * t:32 * (t + 1)]
                nc.scalar.activation(dst, avpre[:].rearrange("p (j b) -> p j b", b=32),
                                     AF.Tanh)
                _flush_stores()

            # ---- remaining output-projection chunks ----
            if t_steps == 48:
                for vt in range(32):
                    _mm_pb(*_load_pb(5, vt))
                    _flush_stores()
            else:
                nch = (CW + 255) // 256
                for n in range(nch):
                    w = min(256, CW - 256 * n)
                    for vt in range(32):
                        _mm_pb(*_load_pb(n, vt), width=w)
                        _flush_stores()

    nc.compile()
    return nc


def _prep(inputs, t_steps=T):
    g = {k: np.asarray(v) for k, v in inputs.items()}
    src = g["src_encodings"].astype(np.float32)          # [S, B, 2E]
    h0 = g["h0"].astype(np.float32)
    c0 = g["c0"].astype(np.float32)
    emb = g["embedding"].astype(np.float32)
    Wp = g["W_proj"].astype(np.float32)
    Wc = g["W_combine"].astype(np.float32)
    Wo = g["W_out"].astype(np.float32)
    Wih0 = g["W_ih0"].astype(np.float32)
    Whh0 = g["W_hh0"].astype(np.float32)
    bih0 = g["b_ih0"].astype(np.float32)
    bhh0 = g["b_hh0"].astype(np.float32)
    Wih1 = g["W_ih1"].astype(np.float32)
    Whh1 = g["W_hh1"].astype(np.float32)
    bih1 = g["b_ih1"].astype(np.float32)
    bhh1 = g["b_hh1"].astype(np.float32)
    tgt = np.asarray(g["tgt_tensor"]).astype(np.int64)   # [T, B]

    W1 = Wih1 + Whh1
    b0 = bih0 + bhh0
    b1 = bih1 + bhh1

    # shared across cores
    wemb = emb[tgt[:t_steps]]                            # [t, B, E]
    # wordt: [128, t*128]; step block t = wordT[:,t] split into 4 j-blocks
    wordt = (wemb.transpose(0, 2, 1)                     # [t, E, B]
             .reshape(t_steps, 4, 128, 32)
             .transpose(2, 0, 1, 3).reshape(128, t_steps * 128))
    wordt = np.ascontiguousarray(wordt).astype(BF_NP)
    enct = np.ascontiguousarray(
        src.transpose(2, 1, 0).reshape(1024, 2048))  # [e, b*64+s]
    h0t = np.ascontiguousarray(
        h0.T.reshape(8, 128, 32).transpose(1, 0, 2).reshape(128, 256)
    ).astype(BF_NP)

    in_maps = []
    for k in range(P):
        rows = np.concatenate([gg * 1024 + k * 128 + np.arange(128)
                               for gg in (0, 1, 3, 2)])  # [i|f|o|g] x 128 dims
        # W0sT_aug rows: [h2 1024 | word 512 | bias 1 | pad | av 1024]
        w0a = np.zeros((NK0 * 128, 512), np.float32)
        w0a[0:1024] = Whh0[rows].T
        w0a[1024:1536] = Wih0[rows, 0:512].T
        w0a[1536] = b0[rows]
        w0a[1664:2688] = Wih0[rows, 512:1536].T
        w0a[:, 0:384] *= 0.5   # i,f,o via tanh: sigma(x) = 0.5*tanh(x/2)+0.5
        w0s = np.ascontiguousarray(
            w0a.reshape(NK0, 128, 512).transpose(1, 0, 2).reshape(128, NK0 * 512)
        ).astype(BF_NP)

        w1a = np.zeros((NK1 * 128, 512), np.float32)
        w1a[0:1024] = W1[rows].T
        w1a[1024] = b1[rows]
        w1a[:, 0:384] *= 0.5
        w1s = np.ascontiguousarray(
            w1a.reshape(NK1, 128, 512).transpose(1, 0, 2).reshape(128, NK1 * 512)
        ).astype(BF_NP)

        # Wc own-K slice: h dims [128k..] and ctx dims [1024+128k..]
        hs = slice(k * 128, k * 128 + 128)
        cs = slice(1024 + k * 128, 1024 + k * 128 + 128)
        wc_own = np.concatenate([Wc[:, hs], Wc[:, cs]], axis=1)  # [1024, 256]
        blocks = []
        for j in range(2):
            for m in range(8):
                blocks.append(wc_own[128 * m:128 * (m + 1),
                                     128 * j:128 * (j + 1)].T)
        wcs = np.ascontiguousarray(
            np.concatenate(blocks, axis=1)).astype(BF_NP)  # [128, 16*128]

        wot = np.ascontiguousarray(
            Wo[VSH * k:VSH * (k + 1)].T).astype(BF_NP)     # [1024, 4000]
        wpt_ = Wp[128 * k:128 * (k + 1), :].T              # [1024, 128]
        wpt = np.ascontiguousarray(
            wpt_.reshape(8, 128, 128).transpose(1, 0, 2).reshape(128, 8 * 128))
        # encse[s, b*128 + e] = src[s, b, e_shard]
        encse = np.ascontiguousarray(
            src[:, :, 128 * k:128 * (k + 1)].reshape(64, 32 * 128)).astype(BF_NP)
        c0s = np.ascontiguousarray(c0[:, 128 * k:128 * (k + 1)])

        in_maps.append({
            "w0s": w0s, "w1s": w1s, "wcs": wcs, "wot": wot, "wpt": wpt,
            "enct": enct, "encse": encse, "wordt": wordt,
            "h0t": h0t, "c0s": c0s,
        })
    return in_maps


_CACHE = {}


def _get_nc(t_steps=T):
    if t_steps not in _CACHE:
        _CACHE[t_steps] = _build(t_steps)
    return _CACHE[t_steps]


def run_device(inputs, trace=False, t_steps=T):
    nc = _get_nc(t_steps)
    in_maps = _prep(inputs, t_steps)
    return run_bass_kernel_spmd(nc, in_maps, core_ids=list(range(P)), trace=trace)


def assemble(results, t_steps=T):
    return np.concatenate(
        [np.ascontiguousarray(np.asarray(results[k]["out"]).T)
         .reshape(t_steps, B, VSH) for k in range(P)],
        axis=2)


def kernel(**inputs):
    r = run_device(inputs)
    return assemble(r.results)
